# revision 1
# baseline (speedup 1.0000x reference)
"""Rolling-mean (window=60) over time axis of (2048, 3000, 8) f32, via
banded-matmul on 8 NeuronCores. Assets axis (3000) sharded 8 ways."""

import numpy as np

import concourse.bacc as bacc
import concourse.mybir as mybir
import concourse.tile as tile
from concourse.bass_utils import run_bass_kernel_spmd

# Problem constants (hardcoded per harness contract)
T = 2048          # time steps
N_FULL = 3000     # assets
J = 8             # characteristics
WIN = 60          # rolling window
N_CORES = 8
S_CORE = N_FULL * J // N_CORES  # 3000 series per core, contiguous in memory
P = 128
N_TILES = T // P  # 16

# free-dim chunks of <=512 (one PSUM bank each)
CHUNKS = [(f0, min(512, S_CORE - f0)) for f0 in range(0, S_CORE, 512)]

MM_DT = mybir.dt.float32r  # 1 cycle/row on PE at free>=256 (vs 4 for float32)


def _band_weights():
    """lhsT-layout (k, m) band matrices with the 1/60 scale folded in.

    out_tile_i[m] = sum_k A[k,m] * x_i[k] + sum_k B[k,m] * x_{i-1}[k]
    A0 replaces A for tile 0 (rows < 59 get the first full window's mean).
    """
    s = np.float32(1.0 / WIN)
    A = np.zeros((P, P), np.float32)
    B = np.zeros((P, P), np.float32)
    A0 = np.zeros((P, P), np.float32)
    for m in range(P):
        A[max(0, m - (WIN - 1)): m + 1, m] = s
        if m < WIN - 1:
            B[m + P - (WIN - 1):, m] = s
            A0[0:WIN, m] = s
        else:
            A0[m - (WIN - 1): m + 1, m] = s
    return np.stack([A0, A, B])  # (3, 128, 128)


def _build_nc():
    nc = bacc.Bacc(
        "TRN2",
        target_bir_lowering=False,
        debug=False,
        num_devices=N_CORES,
    )
    x_dram = nc.dram_tensor("x", [T, S_CORE], MM_DT, kind="ExternalInput").ap()
    w_dram = nc.dram_tensor("w", [3, P, P], MM_DT, kind="ExternalInput").ap()
    y_dram = nc.dram_tensor(
        "y", [T, S_CORE], mybir.dt.float32, kind="ExternalOutput"
    ).ap()

    with tile.TileContext(nc) as tc:
        with (
            tc.tile_pool(name="consts", bufs=1) as cpool,
            tc.tile_pool(name="xin", bufs=4) as xpool,
            tc.tile_pool(name="yout", bufs=3) as ypool,
            tc.tile_pool(name="psum", bufs=8, space="PSUM") as ppool,
        ):
            wA0 = cpool.tile([P, P], MM_DT)
            wA = cpool.tile([P, P], MM_DT)
            wB = cpool.tile([P, P], MM_DT)
            nc.sync.dma_start(out=wA0[:], in_=w_dram[0])
            nc.sync.dma_start(out=wA[:], in_=w_dram[1])
            nc.sync.dma_start(out=wB[:], in_=w_dram[2])

            x_prev = None
            for i in range(N_TILES):
                x_i = xpool.tile([P, S_CORE], MM_DT, tag="x")
                nc.sync.dma_start(out=x_i[:], in_=x_dram[P * i: P * (i + 1), :])
                y_i = ypool.tile([P, S_CORE], mybir.dt.float32, tag="y")
                for f0, fw in CHUNKS:
                    ps = ppool.tile([P, fw], mybir.dt.float32, tag="ps")
                    if i == 0:
                        nc.tensor.matmul(
                            ps[:], wA0[:], x_i[:, f0: f0 + fw],
                            start=True, stop=True,
                        )
                    else:
                        nc.tensor.matmul(
                            ps[:], wA[:], x_i[:, f0: f0 + fw],
                            start=True, stop=False,
                        )
                        nc.tensor.matmul(
                            ps[:], wB[:], x_prev[:, f0: f0 + fw],
                            start=False, stop=True,
                        )
                    nc.vector.tensor_copy(out=y_i[:, f0: f0 + fw], in_=ps[:])
                nc.sync.dma_start(out=y_dram[P * i: P * (i + 1), :], in_=y_i[:])
                x_prev = x_i

    nc.compile()
    return nc


_NC = None


def _get_nc():
    global _NC
    if _NC is None:
        _NC = _build_nc()
    return _NC


def kernel(data, window_size=WIN, **_unused):
    data = np.asarray(data)
    assert data.shape == (T, N_FULL, J), data.shape
    assert int(window_size) == WIN
    in_dtype = data.dtype
    data32 = np.ascontiguousarray(data, dtype=np.float32)

    w = _band_weights()
    nc = _get_nc()

    n_per = N_FULL // N_CORES  # 375 assets per core
    in_maps = []
    for c in range(N_CORES):
        sl = data32[:, c * n_per: (c + 1) * n_per, :].reshape(T, S_CORE)
        in_maps.append({"x": np.ascontiguousarray(sl), "w": w})

    res = run_bass_kernel_spmd(nc, in_maps, core_ids=list(range(N_CORES)))
    parts = [
        res.results[c]["y"].reshape(T, n_per, J) for c in range(N_CORES)
    ]
    out = np.concatenate(parts, axis=1)
    return out.astype(in_dtype, copy=False)
